# revision 6
# baseline (speedup 1.0000x reference)
"""DenseDistance kernel for Trainium2 (8 NeuronCores, SPMD batch-sharded).

out[b, u] = sqrt(max(sum_d (x[b,d] - W[d,u])^2, eps))
          = sqrt(||x_b||^2 + ||w_u||^2 - 2 x_b . w_u)        (values >> eps here)

Sharding: x (2048, 256) split along batch into 8 shards of 256 rows; the
(256, 512) weight matrix is replicated to every core. Each core computes its
(256, 512) output slab; host concatenates.

Host-side input prep (layout + auxiliary scalars; 0.2% of the FLOPs):
  xt  [128, 512]  per-core x shard, transposed to d-major, two 128-d chunks
                  packed along free dim: xt[:, j*256+b] = x[b, j*128+d]
  wm  [128, 1024] -2*W, d-chunk packed:  wm[:, j*512+u] = -2*W[j*128+d, u]
  cc  [1, 512]    column norms  ||w_u||^2
  rr  [128, 2]    row norms ||x_b||^2, column i covers batch rows i*128..+128

Device (per core): for each 128-row batch tile i
  psum = sum_j xt_j.T @ wm_j      (two K=128 f32r matmuls, PSUM accum)
  psum += ones_1x128.T @ cc       (K=1 outer product broadcasts cc)
  out  = Sqrt(psum + rr[:, i])    (ACT, per-partition bias)
All matmuls in float32r (1 col/cycle vs 4 for fp32). DMAs split across the
two HWDGE queues (sync, scalar) so transfers overlap.
"""

import sys

sys.path.insert(0, "/opt/trn_rl_repo")

import numpy as np

B, D, U = 2048, 256, 512
NCORES = 8
BS = B // NCORES  # 256 batch rows per core
P = 128  # SBUF partitions

_cache = {}


def _build():
    from contextlib import ExitStack

    from concourse import bacc, mybir, tile

    F32 = mybir.dt.float32
    F32R = mybir.dt.float32r

    nc = bacc.Bacc(
        "TRN2",
        target_bir_lowering=False,
        debug=False,
        enable_asserts=False,
        num_devices=NCORES,
    )
    xt_d = nc.dram_tensor("xt", [P, 2 * BS], F32R, kind="ExternalInput").ap()
    wm_d = nc.dram_tensor("wm", [P, 2 * U], F32R, kind="ExternalInput").ap()
    # cc packs [ones(128) | colnorms(512)] so the K=1 broadcast matmul's
    # stationary + moving operands arrive in one f32r DMA
    cc_d = nc.dram_tensor("cc", [1, P + U], F32R, kind="ExternalInput").ap()
    rr_d = nc.dram_tensor("rr", [P, 2], F32, kind="ExternalInput").ap()
    out_d = nc.dram_tensor("out", [BS, U], F32, kind="ExternalOutput").ap()

    with tile.TileContext(nc) as tc, ExitStack() as ctx:
        pool = ctx.enter_context(tc.tile_pool(name="sb", bufs=1))
        psum = ctx.enter_context(tc.tile_pool(name="ps", bufs=1, space="PSUM"))

        xt = pool.tile([P, 2 * BS], F32R, name="xt", tag="xt")
        wm = pool.tile([P, 2 * U], F32R, name="wm", tag="wm")
        cc = pool.tile([1, P + U], F32R, name="cc", tag="cc")
        rr = pool.tile([P, 2], F32, name="rr", tag="rr")

        # loads, split per d-chunk across the two HWDGE rings so the first
        # matmul's operands land first and the small cc/rr loads aren't
        # stuck behind 256KB transfers (completion receipt is ~2us each,
        # pipelined within a ring)
        nc.sync.dma_start(xt[:, 0:BS], xt_d[:, 0:BS])
        nc.scalar.dma_start(wm[:, 0:U], wm_d[:, 0:U])
        nc.sync.dma_start(xt[:, BS : 2 * BS], xt_d[:, BS : 2 * BS])
        nc.scalar.dma_start(wm[:, U : 2 * U], wm_d[:, U : 2 * U])
        nc.sync.dma_start(cc[:], cc_d[:])
        nc.sync.dma_start(rr[:], rr_d[:])

        # PE warm-up: the HAM clock gate holds the PE at reduced clock until
        # it has been busy for a few us. Burn dummy bf16 matmuls during the
        # DMA wait so the real matmuls run at full clock. Each is ~128 cols,
        # so overshoot past DMA arrival costs at most ~0.2us.
        BF16 = mybir.dt.bfloat16
        warm_w = pool.tile([P, P], BF16, name="warm_w", tag="warm_w")
        warm_x = pool.tile([P, P], BF16, name="warm_x", tag="warm_x")
        nc.gpsimd.memset(warm_w[:], 1.0)
        nc.gpsimd.memset(warm_x[:], 1.0)
        warm_pm = psum.tile([P, P], F32, name="warm_pm", tag="warm_pm")
        for _ in range(14):
            nc.tensor.matmul(warm_pm[:], warm_w[:], warm_x[:], start=True, stop=True)

        for i in range(2):
            pm = psum.tile([P, U], F32, name=f"pm{i}", tag=f"pm{i}")
            nc.tensor.matmul(
                pm[:],
                xt[:, i * P : (i + 1) * P],
                wm[:, 0:U],
                start=True,
                stop=False,
            )
            nc.tensor.matmul(
                pm[:],
                xt[:, BS + i * P : BS + (i + 1) * P],
                wm[:, U : 2 * U],
                start=False,
                stop=False,
            )
            nc.tensor.matmul(
                pm[:], cc[:, 0:P], cc[:, P : P + U],
                start=False, stop=True,
            )
            res = pool.tile([P, U], F32, name=f"res{i}", tag=f"res{i}")
            nc.scalar.activation(
                res[:],
                pm[:],
                mybir.ActivationFunctionType.Sqrt,
                bias=rr[:, i : i + 1],
            )
            # split each 256KB store across both HWDGE queues
            nc.sync.dma_start(out_d[i * P : (i + 1) * P, 0 : U // 2], res[:, 0 : U // 2])
            nc.scalar.dma_start(
                out_d[i * P : (i + 1) * P, U // 2 : U], res[:, U // 2 : U]
            )

    nc.compile()
    if not nc.is_finalized():
        nc.finalize()
    return nc


def _get_nc():
    if "nc" not in _cache:
        _cache["nc"] = _build()
    return _cache["nc"]


def _prep_inputs(x, w):
    """Host-side shard + layout prep. Returns per-core in_maps."""
    wneg = -2.0 * w  # (D, U)
    wm = np.ascontiguousarray(
        np.concatenate([wneg[0:P], wneg[P : 2 * P]], axis=1)
    )  # [128, 1024]
    cc = np.ascontiguousarray(
        np.concatenate(
            [np.ones(P, dtype=np.float32), (w * w).sum(axis=0, dtype=np.float32)]
        )[None, :]
    )  # [1, P+U]
    in_maps = []
    for c in range(NCORES):
        xs = x[c * BS : (c + 1) * BS]  # (256, 256)
        xT = xs.T  # (D, BS)
        xt = np.ascontiguousarray(np.concatenate([xT[0:P], xT[P : 2 * P]], axis=1))
        rr = np.ascontiguousarray(
            (xs * xs).sum(axis=1, dtype=np.float32).reshape(2, P).T
        )  # [128, 2]
        in_maps.append({"xt": xt, "wm": wm, "cc": cc, "rr": rr})
    return in_maps


def run(inputs, trace=False):
    """Run on 8 cores; returns (full_output, BassKernelResults)."""
    from concourse.bass_utils import run_bass_kernel_spmd

    x = np.ascontiguousarray(np.asarray(inputs["x"], dtype=np.float32))
    w = np.ascontiguousarray(np.asarray(inputs["kernel"], dtype=np.float32))
    assert x.shape == (B, D) and w.shape == (D, U)

    nc = _get_nc()
    in_maps = _prep_inputs(x, w)
    res = run_bass_kernel_spmd(nc, in_maps, list(range(NCORES)), trace=trace)
    out = np.concatenate([res.results[c]["out"] for c in range(NCORES)], axis=0)
    return out, res


def kernel(**inputs):
    out, _ = run(inputs, trace=False)
    return out


# revision 8
# speedup vs baseline: 1.0373x; 1.0373x over previous
"""DenseDistance kernel for Trainium2 (8 NeuronCores, SPMD batch-sharded).

out[b, u] = sqrt(max(sum_d (x[b,d] - W[d,u])^2, eps))
          = sqrt(||x_b||^2 + ||w_u||^2 - 2 x_b . w_u)        (values >> eps here)

Sharding: x (2048, 256) split along batch into 8 shards of 256 rows; the
(256, 512) weight matrix is replicated to every core. Each core computes its
(256, 512) output slab; host concatenates.

Host-side input prep (layout + auxiliary scalars; 0.2% of the FLOPs):
  xt  [128, 512]  per-core x shard, transposed to d-major, two 128-d chunks
                  packed along free dim: xt[:, j*256+b] = x[b, j*128+d]
  wm  [128, 1024] -2*W, d-chunk packed:  wm[:, j*512+u] = -2*W[j*128+d, u]
  cc  [1, 512]    column norms  ||w_u||^2
  rr  [128, 2]    row norms ||x_b||^2, column i covers batch rows i*128..+128

Device (per core): for each 128-row batch tile i
  psum = sum_j xt_j.T @ wm_j      (two K=128 f32r matmuls, PSUM accum)
  psum += ones_1x128.T @ cc       (K=1 outer product broadcasts cc)
  out  = Sqrt(psum + rr[:, i])    (ACT, per-partition bias)
All matmuls in float32r (1 col/cycle vs 4 for fp32). DMAs split across the
two HWDGE queues (sync, scalar) so transfers overlap.
"""

import sys

sys.path.insert(0, "/opt/trn_rl_repo")

import numpy as np

B, D, U = 2048, 256, 512
NCORES = 8
BS = B // NCORES  # 256 batch rows per core
P = 128  # SBUF partitions

_cache = {}


def _build():
    from contextlib import ExitStack

    from concourse import bacc, mybir, tile

    F32 = mybir.dt.float32
    F32R = mybir.dt.float32r

    nc = bacc.Bacc(
        "TRN2",
        target_bir_lowering=False,
        debug=False,
        enable_asserts=False,
        num_devices=NCORES,
    )
    xt_d = nc.dram_tensor("xt", [P, 2 * BS], F32R, kind="ExternalInput").ap()
    wm_d = nc.dram_tensor("wm", [P, 2 * U], F32R, kind="ExternalInput").ap()
    # cc packs [ones(128) | colnorms(512)] so the K=1 broadcast matmul's
    # stationary + moving operands arrive in one f32r DMA
    cc_d = nc.dram_tensor("cc", [1, P + U], F32R, kind="ExternalInput").ap()
    rr_d = nc.dram_tensor("rr", [P, 2], F32, kind="ExternalInput").ap()
    out_d = nc.dram_tensor("out", [BS, U], F32, kind="ExternalOutput").ap()

    with tile.TileContext(nc) as tc, ExitStack() as ctx:
        pool = ctx.enter_context(tc.tile_pool(name="sb", bufs=1))
        psum = ctx.enter_context(tc.tile_pool(name="ps", bufs=1, space="PSUM"))

        xt = pool.tile([P, 2 * BS], F32R, name="xt", tag="xt")
        wm = pool.tile([P, 2 * U], F32R, name="wm", tag="wm")
        cc = pool.tile([1, P + U], F32R, name="cc", tag="cc")
        rr = pool.tile([P, 2], F32, name="rr", tag="rr")

        # loads: DMA completion receipts (~2us) serialize within a ring, so
        # keep exactly ONE big input DMA per HWDGE ring and push the tiny
        # cc/rr loads to the gpsimd SWDGE ring (a third, independent channel)
        nc.sync.dma_start(xt[:], xt_d[:])
        nc.scalar.dma_start(wm[:], wm_d[:])
        nc.gpsimd.dma_start(cc[:], cc_d[:])
        nc.gpsimd.dma_start(rr[:], rr_d[:])

        # PE warm-up: the HAM clock gate holds the PE at reduced clock until
        # it has been busy for a few us. Burn dummy bf16 matmuls during the
        # DMA wait so the real matmuls run at full clock. They pipeline at
        # ~110ns apiece, filling the ~3.5us until the wm DMA lands.
        BF16 = mybir.dt.bfloat16
        warm_w = pool.tile([P, P], BF16, name="warm_w", tag="warm_w")
        warm_x = pool.tile([P, P], BF16, name="warm_x", tag="warm_x")
        nc.gpsimd.memset(warm_w[:], 1.0)
        nc.gpsimd.memset(warm_x[:], 1.0)
        warm_pm = psum.tile([P, P], F32, name="warm_pm", tag="warm_pm")
        for _ in range(28):
            nc.tensor.matmul(warm_pm[:], warm_w[:], warm_x[:], start=True, stop=True)

        for i in range(2):
            pm = psum.tile([P, U], F32, name=f"pm{i}", tag=f"pm{i}")
            nc.tensor.matmul(
                pm[:],
                xt[:, i * P : (i + 1) * P],
                wm[:, 0:U],
                start=True,
                stop=False,
            )
            nc.tensor.matmul(
                pm[:],
                xt[:, BS + i * P : BS + (i + 1) * P],
                wm[:, U : 2 * U],
                start=False,
                stop=False,
            )
            nc.tensor.matmul(
                pm[:], cc[:, 0:P], cc[:, P : P + U],
                start=False, stop=True,
            )
            res = pool.tile([P, U], F32, name=f"res{i}", tag=f"res{i}")
            nc.scalar.activation(
                res[:],
                pm[:],
                mybir.ActivationFunctionType.Sqrt,
                bias=rr[:, i : i + 1],
            )
            # one whole-tile store per ring (receipts serialize per ring)
            eng = nc.sync if i == 0 else nc.scalar
            eng.dma_start(out_d[i * P : (i + 1) * P, :], res[:])

    nc.compile()
    if not nc.is_finalized():
        nc.finalize()
    return nc


def _get_nc():
    if "nc" not in _cache:
        _cache["nc"] = _build()
    return _cache["nc"]


def _prep_inputs(x, w):
    """Host-side shard + layout prep. Returns per-core in_maps."""
    wneg = -2.0 * w  # (D, U)
    wm = np.ascontiguousarray(
        np.concatenate([wneg[0:P], wneg[P : 2 * P]], axis=1)
    )  # [128, 1024]
    cc = np.ascontiguousarray(
        np.concatenate(
            [np.ones(P, dtype=np.float32), (w * w).sum(axis=0, dtype=np.float32)]
        )[None, :]
    )  # [1, P+U]
    in_maps = []
    for c in range(NCORES):
        xs = x[c * BS : (c + 1) * BS]  # (256, 256)
        xT = xs.T  # (D, BS)
        xt = np.ascontiguousarray(np.concatenate([xT[0:P], xT[P : 2 * P]], axis=1))
        rr = np.ascontiguousarray(
            (xs * xs).sum(axis=1, dtype=np.float32).reshape(2, P).T
        )  # [128, 2]
        in_maps.append({"xt": xt, "wm": wm, "cc": cc, "rr": rr})
    return in_maps


def run(inputs, trace=False):
    """Run on 8 cores; returns (full_output, BassKernelResults)."""
    from concourse.bass_utils import run_bass_kernel_spmd

    x = np.ascontiguousarray(np.asarray(inputs["x"], dtype=np.float32))
    w = np.ascontiguousarray(np.asarray(inputs["kernel"], dtype=np.float32))
    assert x.shape == (B, D) and w.shape == (D, U)

    nc = _get_nc()
    in_maps = _prep_inputs(x, w)
    res = run_bass_kernel_spmd(nc, in_maps, list(range(NCORES)), trace=trace)
    out = np.concatenate([res.results[c]["out"] for c in range(NCORES)], axis=0)
    return out, res


def kernel(**inputs):
    out, _ = run(inputs, trace=False)
    return out


# revision 9
# speedup vs baseline: 1.1402x; 1.0992x over previous
"""DenseDistance kernel for Trainium2 (8 NeuronCores, SPMD batch-sharded).

out[b, u] = sqrt(max(sum_d (x[b,d] - W[d,u])^2, eps))
          = sqrt(||x_b||^2 + ||w_u||^2 - 2 x_b . w_u)        (values >> eps here)

Sharding: x (2048, 256) split along batch into 8 shards of 256 rows; the
(256, 512) weight matrix is replicated to every core. Each core computes its
(256, 512) output slab; host concatenates.

Host-side input prep (layout + auxiliary scalars; 0.2% of the FLOPs):
  xt  [128, 512]  per-core x shard, transposed to d-major, two 128-d chunks
                  packed along free dim: xt[:, j*256+b] = x[b, j*128+d]
  wm  [128, 1024] -2*W, d-chunk packed:  wm[:, j*512+u] = -2*W[j*128+d, u]
  cc  [1, 512]    column norms  ||w_u||^2
  rr  [128, 2]    row norms ||x_b||^2, column i covers batch rows i*128..+128

Device (per core): for each 128-row batch tile i
  psum = sum_j xt_j.T @ wm_j      (two K=128 f32r matmuls, PSUM accum)
  psum += ones_1x128.T @ cc       (K=1 outer product broadcasts cc)
  out  = Sqrt(psum + rr[:, i])    (ACT, per-partition bias)
All matmuls in float32r (1 col/cycle vs 4 for fp32). DMAs split across the
two HWDGE queues (sync, scalar) so transfers overlap.
"""

import sys

sys.path.insert(0, "/opt/trn_rl_repo")

import numpy as np

B, D, U = 2048, 256, 512
NCORES = 8
BS = B // NCORES  # 256 batch rows per core
P = 128  # SBUF partitions

_cache = {}


def _build():
    from contextlib import ExitStack

    from concourse import bacc, mybir, tile

    F32 = mybir.dt.float32
    F32R = mybir.dt.float32r

    nc = bacc.Bacc(
        "TRN2",
        target_bir_lowering=False,
        debug=False,
        enable_asserts=False,
        num_devices=NCORES,
    )
    BF16 = mybir.dt.bfloat16
    xt_d = nc.dram_tensor("xt", [P, 2 * BS], BF16, kind="ExternalInput").ap()
    wm_d = nc.dram_tensor("wm", [P, 2 * U], BF16, kind="ExternalInput").ap()
    # cc packs [ones(128) | colnorms(512)] so the K=1 broadcast matmul's
    # stationary + moving operands arrive in one f32r DMA
    cc_d = nc.dram_tensor("cc", [1, P + U], F32R, kind="ExternalInput").ap()
    rr_d = nc.dram_tensor("rr", [P, 2], F32, kind="ExternalInput").ap()
    out_d = nc.dram_tensor("out", [BS, U], F32, kind="ExternalOutput").ap()

    with tile.TileContext(nc) as tc, ExitStack() as ctx:
        pool = ctx.enter_context(tc.tile_pool(name="sb", bufs=1))
        psum = ctx.enter_context(tc.tile_pool(name="ps", bufs=1, space="PSUM"))

        xt = pool.tile([P, 2 * BS], BF16, name="xt", tag="xt")
        wm = pool.tile([P, 2 * U], BF16, name="wm", tag="wm")
        cc = pool.tile([1, P + U], F32R, name="cc", tag="cc")
        rr = pool.tile([P, 2], F32, name="rr", tag="rr")

        # loads: DMA completion receipts (~2us) serialize within a ring, so
        # keep exactly ONE big input DMA per HWDGE ring and push the tiny
        # cc/rr loads to the gpsimd SWDGE ring (a third, independent channel)
        nc.sync.dma_start(xt[:], xt_d[:])
        nc.scalar.dma_start(wm[:], wm_d[:])
        nc.gpsimd.dma_start(rr[:], rr_d[:])
        nc.gpsimd.dma_start(cc[:], cc_d[:])

        # PE warm-up: the HAM clock gate holds the PE at reduced clock until
        # it has been busy for a few us. Burn dummy bf16 matmuls during the
        # DMA wait so the real matmuls run at full clock. They pipeline at
        # ~110ns apiece, filling the ~3.5us until the wm DMA lands.
        warm_w = pool.tile([P, P], BF16, name="warm_w", tag="warm_w")
        warm_x = pool.tile([P, P], BF16, name="warm_x", tag="warm_x")
        nc.vector.memset(warm_w[:], 1.0)
        nc.vector.memset(warm_x[:], 1.0)
        warm_pm = psum.tile([P, P], F32, name="warm_pm", tag="warm_pm")
        for _ in range(38):
            nc.tensor.matmul(warm_pm[:], warm_w[:], warm_x[:], start=True, stop=True)

        for i in range(2):
            pm = psum.tile([P, U], F32, name=f"pm{i}", tag=f"pm{i}")
            nc.tensor.matmul(
                pm[:],
                xt[:, i * P : (i + 1) * P],
                wm[:, 0:U],
                start=True,
                stop=False,
            )
            nc.tensor.matmul(
                pm[:],
                xt[:, BS + i * P : BS + (i + 1) * P],
                wm[:, U : 2 * U],
                start=False,
                stop=False,
            )
            nc.tensor.matmul(
                pm[:], cc[:, 0:P], cc[:, P : P + U],
                start=False, stop=True,
            )
            res = pool.tile([P, U], F32, name=f"res{i}", tag=f"res{i}")
            nc.scalar.activation(
                res[:],
                pm[:],
                mybir.ActivationFunctionType.Sqrt,
                bias=rr[:, i : i + 1],
            )
            # one whole-tile store per ring (receipts serialize per ring)
            eng = nc.sync if i == 0 else nc.scalar
            eng.dma_start(out_d[i * P : (i + 1) * P, :], res[:])

    nc.compile()
    if not nc.is_finalized():
        nc.finalize()
    return nc


def _get_nc():
    if "nc" not in _cache:
        _cache["nc"] = _build()
    return _cache["nc"]


def _prep_inputs(x, w):
    """Host-side shard + layout prep. Returns per-core in_maps."""
    import ml_dtypes

    bf16 = ml_dtypes.bfloat16
    wneg = -2.0 * w  # (D, U)
    wm = np.ascontiguousarray(
        np.concatenate([wneg[0:P], wneg[P : 2 * P]], axis=1).astype(bf16)
    )  # [128, 1024]
    cc = np.ascontiguousarray(
        np.concatenate(
            [np.ones(P, dtype=np.float32), (w * w).sum(axis=0, dtype=np.float32)]
        )[None, :]
    )  # [1, P+U]
    in_maps = []
    for c in range(NCORES):
        xs = x[c * BS : (c + 1) * BS]  # (256, 256)
        xT = xs.T  # (D, BS)
        xt = np.ascontiguousarray(
            np.concatenate([xT[0:P], xT[P : 2 * P]], axis=1).astype(bf16)
        )
        rr = np.ascontiguousarray(
            (xs * xs).sum(axis=1, dtype=np.float32).reshape(2, P).T
        )  # [128, 2]
        in_maps.append({"xt": xt, "wm": wm, "cc": cc, "rr": rr})
    return in_maps


def run(inputs, trace=False):
    """Run on 8 cores; returns (full_output, BassKernelResults)."""
    from concourse.bass_utils import run_bass_kernel_spmd

    x = np.ascontiguousarray(np.asarray(inputs["x"], dtype=np.float32))
    w = np.ascontiguousarray(np.asarray(inputs["kernel"], dtype=np.float32))
    assert x.shape == (B, D) and w.shape == (D, U)

    nc = _get_nc()
    in_maps = _prep_inputs(x, w)
    res = run_bass_kernel_spmd(nc, in_maps, list(range(NCORES)), trace=trace)
    out = np.concatenate([res.results[c]["out"] for c in range(NCORES)], axis=0)
    return out, res


def kernel(**inputs):
    out, _ = run(inputs, trace=False)
    return out


# revision 10
# speedup vs baseline: 1.2566x; 1.1021x over previous
"""DenseDistance kernel for Trainium2 (8 NeuronCores, SPMD batch-sharded).

out[b, u] = sqrt(max(sum_d (x[b,d] - W[d,u])^2, eps))
          = sqrt(||x_b||^2 + ||w_u||^2 - 2 x_b . w_u)        (values >> eps here)

Sharding: x (2048, 256) split along batch into 8 shards of 256 rows; the
(256, 512) weight matrix is replicated to every core. Each core computes its
(256, 512) output slab; host concatenates.

Host-side input prep (layout + auxiliary scalars; 0.2% of the FLOPs):
  xt  [128, 512]  per-core x shard, transposed to d-major, two 128-d chunks
                  packed along free dim: xt[:, j*256+b] = x[b, j*128+d]
  wm  [128, 1024] -2*W, d-chunk packed:  wm[:, j*512+u] = -2*W[j*128+d, u]
  cc  [1, 512]    column norms  ||w_u||^2
  rr  [128, 2]    row norms ||x_b||^2, column i covers batch rows i*128..+128

Device (per core): for each 128-row batch tile i
  psum = sum_j xt_j.T @ wm_j      (two K=128 f32r matmuls, PSUM accum)
  psum += ones_1x128.T @ cc       (K=1 outer product broadcasts cc)
  out  = Sqrt(psum + rr[:, i])    (ACT, per-partition bias)
All matmuls in float32r (1 col/cycle vs 4 for fp32). DMAs split across the
two HWDGE queues (sync, scalar) so transfers overlap.
"""

import sys

sys.path.insert(0, "/opt/trn_rl_repo")

import numpy as np

B, D, U = 2048, 256, 512
NCORES = 8
BS = B // NCORES  # 256 batch rows per core
P = 128  # SBUF partitions

_cache = {}


def _build():
    from contextlib import ExitStack

    from concourse import bacc, mybir, tile

    F32 = mybir.dt.float32
    F32R = mybir.dt.float32r

    nc = bacc.Bacc(
        "TRN2",
        target_bir_lowering=False,
        debug=False,
        enable_asserts=False,
        num_devices=NCORES,
    )
    BF16 = mybir.dt.bfloat16
    # xw = [xt | wm] concatenated: one 384KB bf16 DMA on the sync ring
    xw_d = nc.dram_tensor("xw", [P, 2 * BS + 2 * U], BF16, kind="ExternalInput").ap()
    # cc packs [ones(128) | colnorms(512)] so the K=1 broadcast matmul's
    # stationary + moving operands arrive in one f32r DMA
    cc_d = nc.dram_tensor("cc", [1, P + U], F32R, kind="ExternalInput").ap()
    rr_d = nc.dram_tensor("rr", [P, 2], F32, kind="ExternalInput").ap()
    out_d = nc.dram_tensor("out", [BS, U], F32, kind="ExternalOutput").ap()

    with tile.TileContext(nc) as tc, ExitStack() as ctx:
        pool = ctx.enter_context(tc.tile_pool(name="sb", bufs=1))
        psum = ctx.enter_context(tc.tile_pool(name="ps", bufs=1, space="PSUM"))

        xw = pool.tile([P, 2 * BS + 2 * U], BF16, name="xw", tag="xw")
        cc = pool.tile([1, P + U], F32R, name="cc", tag="cc")
        rr = pool.tile([P, 2], F32, name="rr", tag="rr")
        xt = xw[:, 0 : 2 * BS]
        wm = xw[:, 2 * BS : 2 * BS + 2 * U]

        # ONE big input DMA on the sync ring (receipts serialize per ring,
        # and the scalar ring is polluted by the ACT table-load DMAs);
        # tiny cc/rr ride the gpsimd SWDGE ring
        nc.sync.dma_start(xw[:], xw_d[:])
        nc.gpsimd.dma_start(cc[:], cc_d[:])
        nc.gpsimd.dma_start(rr[:], rr_d[:])

        # dummy Sqrt on a const tile: pulls the Sqrt act-table load (a ~1.3us
        # DMA) to kernel start instead of just before the first real Sqrt
        warm_b = pool.tile([P, 1], F32, name="warm_b", tag="warm_b")
        nc.vector.memset(warm_b[:], 1.0)
        warm_s = pool.tile([P, 1], F32, name="warm_s", tag="warm_s")
        nc.scalar.activation(
            warm_s[:], warm_b[:], mybir.ActivationFunctionType.Sqrt, bias=0.0
        )

        # PE warm-up: the HAM clock gate holds the PE at reduced clock until
        # it has been busy for a few us. Burn dummy bf16 matmuls during the
        # DMA wait so the real matmuls run at full clock. They pipeline at
        # ~110ns apiece, filling the ~3.5us until the wm DMA lands.
        warm_w = pool.tile([P, P], BF16, name="warm_w", tag="warm_w")
        warm_x = pool.tile([P, P], BF16, name="warm_x", tag="warm_x")
        nc.vector.memset(warm_w[:], 1.0)
        nc.vector.memset(warm_x[:], 1.0)
        warm_pm = psum.tile([P, P], F32, name="warm_pm", tag="warm_pm")
        for _ in range(30):
            nc.tensor.matmul(warm_pm[:], warm_w[:], warm_x[:], start=True, stop=True)

        for i in range(2):
            pm = psum.tile([P, U], F32, name=f"pm{i}", tag=f"pm{i}")
            # broadcast-c first: only needs cc, which lands before wm, so it
            # runs on the warmed PE while the big input DMA is still in flight
            nc.tensor.matmul(
                pm[:], cc[:, 0:P], cc[:, P : P + U],
                start=True, stop=False,
            )
            nc.tensor.matmul(
                pm[:],
                xt[:, i * P : (i + 1) * P],
                wm[:, 0:U],
                start=False,
                stop=False,
            )
            nc.tensor.matmul(
                pm[:],
                xt[:, BS + i * P : BS + (i + 1) * P],
                wm[:, U : 2 * U],
                start=False,
                stop=True,
            )
            res = pool.tile([P, U], F32, name=f"res{i}", tag=f"res{i}")
            nc.scalar.activation(
                res[:],
                pm[:],
                mybir.ActivationFunctionType.Sqrt,
                bias=rr[:, i : i + 1],
            )
            # one whole-tile store per ring; scalar ring is clean by now
            eng = nc.scalar if i == 0 else nc.sync
            eng.dma_start(out_d[i * P : (i + 1) * P, :], res[:])

    nc.compile()
    if not nc.is_finalized():
        nc.finalize()
    return nc


def _get_nc():
    if "nc" not in _cache:
        _cache["nc"] = _build()
    return _cache["nc"]


def _prep_inputs(x, w):
    """Host-side shard + layout prep. Returns per-core in_maps."""
    import ml_dtypes

    bf16 = ml_dtypes.bfloat16
    wneg = -2.0 * w  # (D, U)
    wm = np.ascontiguousarray(
        np.concatenate([wneg[0:P], wneg[P : 2 * P]], axis=1).astype(bf16)
    )  # [128, 1024]
    cc = np.ascontiguousarray(
        np.concatenate(
            [np.ones(P, dtype=np.float32), (w * w).sum(axis=0, dtype=np.float32)]
        )[None, :]
    )  # [1, P+U]
    in_maps = []
    for c in range(NCORES):
        xs = x[c * BS : (c + 1) * BS]  # (256, 256)
        xT = xs.T  # (D, BS)
        xt = np.concatenate([xT[0:P], xT[P : 2 * P]], axis=1).astype(bf16)
        xw = np.ascontiguousarray(np.concatenate([xt, wm], axis=1))
        rr = np.ascontiguousarray(
            (xs * xs).sum(axis=1, dtype=np.float32).reshape(2, P).T
        )  # [128, 2]
        in_maps.append({"xw": xw, "cc": cc, "rr": rr})
    return in_maps


def run(inputs, trace=False):
    """Run on 8 cores; returns (full_output, BassKernelResults)."""
    from concourse.bass_utils import run_bass_kernel_spmd

    x = np.ascontiguousarray(np.asarray(inputs["x"], dtype=np.float32))
    w = np.ascontiguousarray(np.asarray(inputs["kernel"], dtype=np.float32))
    assert x.shape == (B, D) and w.shape == (D, U)

    nc = _get_nc()
    in_maps = _prep_inputs(x, w)
    res = run_bass_kernel_spmd(nc, in_maps, list(range(NCORES)), trace=trace)
    out = np.concatenate([res.results[c]["out"] for c in range(NCORES)], axis=0)
    return out, res


def kernel(**inputs):
    out, _ = run(inputs, trace=False)
    return out
